# revision 12
# baseline (speedup 1.0000x reference)
"""Trainium2 Bass kernel for nn_CNF_ODE (dense MLP fwd + Hutchinson divergence).

Contract: kernel(**inputs) takes the FULL unsharded inputs (as produced by
setup_inputs) and returns the full output tuple (dy [N,3], -div [N,1]).

Strategy: pure data parallel over the batch axis across 8 NeuronCores.
Host-side we transpose y/v to feature-major [3, N] (and transpose outputs
back) so the device kernel works entirely in feature-major layout with
contiguous DMAs; the tiny MLP weights are replicated (prepared host-side,
including the transposed weight copies the backward pass needs and the
t-folded first-layer bias).

Device math (feature-major, per batch column):
  z1 = W1[:3]T yT + (b1 + t W1[3]);  h1 = silu(z1); d1 = silu'(z1)
  z2 = W2T h1 + b2;                  h2 = silu(z2); d2 = silu'(z2)
  z3 = W3T h2 + b3;                  h3 = silu(z3); d3 = silu'(z3)
  dy = [W4T h3 + b4; 0]
  u3 = (W4 v[:2]T) * d3
  u2 = (W3 u3) * d2
  p  = (W1[:3]T vT) * d1 * (W2 u2)
  -div = -sum_m p[m]          (column sum via matmul with -1s)

Perf notes:
 - matmuls run in bf16 (fp32 matmul costs 2 HW passes); z accumulation in
   fp32 PSUM; silu/silu' evaluated from the fp32 z.
 - silu and derivative_silu live in different ACT table sets (~2.7us per
   switch), so tiles are processed in groups of G: forward (silu) phase for
   all G tiles, then backward (derivative_silu) phase; z is staged to SBUF
   in fp32 by the DVE so PSUM banks recycle quickly.
"""

import os
from contextlib import ExitStack

import ml_dtypes
import numpy as np

import concourse.bacc as bacc
import concourse.bass as bass
import concourse.mybir as mybir
import concourse.tile as tile
from concourse.bass_utils import run_bass_kernel_spmd
from concourse.tile_rust import add_dep_helper

AF = mybir.ActivationFunctionType
OP = mybir.AluOpType
F32 = mybir.dt.float32
BF16 = mybir.dt.bfloat16

N_CORES = 8
N_FULL = 1048576
NCC = N_FULL // N_CORES  # columns per core
F = 512                  # free-dim tile (one PSUM bank of fp32)
G = 16                   # tiles per table-set phase group

# matmul dtype config ("bf16" or "f32") for forward / backward chains
FWD_DT = BF16
BWD_DT = BF16

_BUILD_CACHE = {}
LAST_RESULTS = None  # BassKernelResults of the most recent run (for test.py)


def _build(ncc=NCC, f=F, g=G):
    key = (ncc, f, g, FWD_DT, BWD_DT)
    if key in _BUILD_CACHE:
        return _BUILD_CACHE[key]

    nc = bacc.Bacc(
        "TRN2",
        target_bir_lowering=False,
        debug=False,
        enable_asserts=False,
        num_devices=N_CORES,
    )
    fdt, bdt = FWD_DT, BWD_DT

    yT_d = nc.dram_tensor("yT", [3, ncc], fdt, kind="ExternalInput").ap()
    vT_d = nc.dram_tensor("vT", [3, ncc], bdt, kind="ExternalInput").ap()
    dyT_d = nc.dram_tensor("dyT", [2, ncc], F32, kind="ExternalOutput").ap()
    nd_d = nc.dram_tensor("ndiv", [1, ncc], F32, kind="ExternalOutput").ap()

    cshape = {
        "Wl1": ([3, 128], fdt), "W2": ([128, 128], fdt),
        "W3": ([128, 64], fdt), "W4g": ([128, 2], fdt),
        "W4T": ([2, 64], bdt), "W3Tg": ([128, 128], bdt),
        "W2T": ([128, 128], bdt), "Wl1b": ([3, 128], bdt),
        "nones": ([128, 1], bdt),
        "b1e": ([128, 1], F32), "b2": ([128, 1], F32),
        "b3g": ([128, 1], F32), "b4": ([2, 1], F32),
    }
    cdram = {k: nc.dram_tensor("c_" + k, s, dt, kind="ExternalInput").ap()
             for k, (s, dt) in cshape.items()}

    ntiles = ncc // f
    assert ntiles * f == ncc and ntiles % g == 0

    def _act(phase_list, *args, **kw):
        ins = nc.scalar.activation(*args, **kw)
        if phase_list is not None:
            phase_list.append(ins)
        return ins

    with tile.TileContext(nc) as tc, ExitStack() as ctx:
        consts = ctx.enter_context(tc.tile_pool(name="consts", bufs=1))
        io = ctx.enter_context(tc.tile_pool(name="io", bufs=3))
        work = ctx.enter_context(tc.tile_pool(name="work", bufs=3))
        zst = ctx.enter_context(tc.tile_pool(name="zst", bufs=g // 2 + 2))
        pp = ctx.enter_context(tc.tile_pool(name="pp", bufs=1, space="PSUM"))

        cs = {k: consts.tile(s, dt, name="c" + k, tag="c" + k)
              for k, (s, dt) in cshape.items()}
        for k in cshape:
            nc.sync.dma_start(out=cs[k], in_=cdram[k])

        prev_phase = []
        f2 = 2 * f
        for grp in range(ntiles // g):
            fwd_acts, bwd_acts = [], []
            zs = []
            # ---------- forward phase: silu table set (tile pairs) ----------
            for j in range(0, g, 2):
                it = grp * g + j
                sl = slice(it * f, it * f + f2)
                lo, hi = slice(0, f), slice(f, f2)

                yt = io.tile([3, f2], fdt, tag="yt", bufs=4)
                vt = io.tile([3, f2], bdt, tag="vt", bufs=g // 2 + 2)
                nc.sync.dma_start(out=yt, in_=yT_d[:, sl])
                nc.sync.dma_start(out=vt, in_=vT_d[:, sl])

                z1 = pp.tile([128, f2], F32, tag="fw", bufs=2)
                nc.tensor.matmul(out=z1[:, lo], lhsT=cs["Wl1"], rhs=yt[:, lo])
                nc.tensor.matmul(out=z1[:, hi], lhsT=cs["Wl1"], rhs=yt[:, hi])
                h1 = work.tile([128, f2], fdt, tag="h1")
                _act(fwd_acts, h1, z1, AF.Silu, bias=cs["b1e"])
                zs1 = zst.tile([128, f2], F32, tag="zs1", bufs=g // 2 + 2)
                nc.vector.tensor_scalar_add(zs1, z1, cs["b1e"])

                z2 = pp.tile([128, f2], F32, tag="fw", bufs=2)
                nc.tensor.matmul(out=z2[:, lo], lhsT=cs["W2"], rhs=h1[:, lo])
                nc.tensor.matmul(out=z2[:, hi], lhsT=cs["W2"], rhs=h1[:, hi])
                h2 = work.tile([128, f2], fdt, tag="h2")
                _act(fwd_acts, h2, z2, AF.Silu, bias=cs["b2"])
                zs2 = zst.tile([128, f2], F32, tag="zs2", bufs=g // 2 + 2)
                nc.vector.tensor_scalar_add(zs2, z2, cs["b2"])

                # layer 3 row-packed: pair tile A in psum rows 0-63, B in
                # rows 64-127, same columns -> one bank, concurrent matmuls
                # (disjoint PE column groups), and [128, f] ACT/DVE ops.
                z3 = pp.tile([128, f2], F32, tag="fw", bufs=2)
                nc.tensor.matmul(out=z3[0:64, lo], lhsT=cs["W3"], rhs=h2[:, lo],
                                 tile_position=(0, 0))
                nc.tensor.matmul(out=z3[64:128, lo], lhsT=cs["W3"], rhs=h2[:, hi],
                                 tile_position=(0, 64))
                h3 = work.tile([128, f], fdt, tag="h3")
                _act(fwd_acts, h3, z3[:, lo], AF.Silu, bias=cs["b3g"])
                zs3 = zst.tile([128, f], F32, tag="zs3", bufs=g // 2 + 2)
                nc.vector.tensor_scalar_add(zs3, z3[:, lo], cs["b3g"])

                z4 = pp.tile([128, f2], F32, tag="fw", bufs=2)
                nc.tensor.matmul(out=z4[0:2, lo], lhsT=cs["W4g"][0:64, :],
                                 rhs=h3[0:64, :], tile_position=(0, 0))
                nc.tensor.matmul(out=z4[0:2, hi], lhsT=cs["W4g"][64:128, :],
                                 rhs=h3[64:128, :], tile_position=(64, 0))
                dy_t = io.tile([2, f2], F32, tag="dy", bufs=3)
                nc.vector.tensor_scalar_add(dy_t, z4[0:2, :], cs["b4"])
                nc.sync.dma_start(out=dyT_d[:, sl], in_=dy_t)

                zs.append((vt, zs1, zs2, zs3, sl))

            # ------- backward phase: derivative_silu table set (pairs) -------
            for pj in range(g // 2):
                vt, zs1, zs2, zs3, sl = zs[pj]
                lo, hi = slice(0, f), slice(f, f2)

                d3 = work.tile([128, f], bdt, tag="d3")
                _act(bwd_acts, d3, zs3, AF.Derivative_silu)
                wps = pp.tile([128, f2], F32, tag="bw", bufs=2)
                nc.tensor.matmul(out=wps[0:64, lo], lhsT=cs["W4T"],
                                 rhs=vt[0:2, lo], tile_position=(0, 0))
                nc.tensor.matmul(out=wps[64:128, lo], lhsT=cs["W4T"],
                                 rhs=vt[0:2, hi], tile_position=(0, 64))
                u3 = work.tile([128, f], bdt, tag="u3")
                nc.vector.tensor_mul(u3, d3, wps[:, lo])

                u2ps = pp.tile([128, f2], F32, tag="bw", bufs=2)
                nc.tensor.matmul(out=u2ps[:, lo], lhsT=cs["W3Tg"][0:64, :],
                                 rhs=u3[0:64, :], tile_position=(0, 0))
                nc.tensor.matmul(out=u2ps[:, hi], lhsT=cs["W3Tg"][64:128, :],
                                 rhs=u3[64:128, :], tile_position=(64, 0))
                d2 = work.tile([128, f2], bdt, tag="d2")
                _act(bwd_acts, d2, zs2, AF.Derivative_silu)
                u2 = work.tile([128, f2], bdt, tag="u2")
                nc.vector.tensor_mul(u2, d2, u2ps)

                u1ps = pp.tile([128, f2], F32, tag="bw", bufs=2)
                nc.tensor.matmul(out=u1ps[:, lo], lhsT=cs["W2T"], rhs=u2[:, lo])
                nc.tensor.matmul(out=u1ps[:, hi], lhsT=cs["W2T"], rhs=u2[:, hi])
                d1 = work.tile([128, f2], bdt, tag="d1")
                _act(bwd_acts, d1, zs1, AF.Derivative_silu)

                vps = pp.tile([128, f2], F32, tag="bw", bufs=2)
                nc.tensor.matmul(out=vps[:, lo], lhsT=cs["Wl1b"], rhs=vt[:, lo])
                nc.tensor.matmul(out=vps[:, hi], lhsT=cs["Wl1b"], rhs=vt[:, hi])
                vd = work.tile([128, f2], bdt, tag="vd")
                nc.vector.tensor_mul(vd, d1, vps)
                p = work.tile([128, f2], bdt, tag="p")
                nc.vector.tensor_mul(p, vd, u1ps)

                dv = pp.tile([128, f2], F32, tag="bw", bufs=2)
                nc.tensor.matmul(out=dv[0:1, lo], lhsT=cs["nones"], rhs=p[:, lo])
                nc.tensor.matmul(out=dv[0:1, hi], lhsT=cs["nones"], rhs=p[:, hi])
                nd_t = io.tile([1, f2], F32, tag="nd", bufs=3)
                _act(None, nd_t, dv[0:1, :], AF.Copy)
                nc.sync.dma_start(out=nd_d[:, sl], in_=nd_t)

            # table-set phase purity (ACT-stream ordering only, no sems)
            for a in fwd_acts:
                for b in prev_phase:
                    add_dep_helper(a.ins, b.ins, sync=False, reason="actset")
            for b in bwd_acts:
                for a in fwd_acts:
                    add_dep_helper(b.ins, a.ins, sync=False, reason="actset")
            prev_phase = bwd_acts

    nc.compile()
    _BUILD_CACHE[key] = nc
    return nc


def _np_dt(dt):
    return ml_dtypes.bfloat16 if dt == BF16 else np.float32


def _prep_consts(t, W1, b1, W2, b2, W3, b3, W4, b4):
    fdt, bdt = _np_dt(FWD_DT), _np_dt(BWD_DT)
    c = {
        "Wl1": (W1[:3, :], fdt), "W2": (W2, fdt), "W3": (W3, fdt),
        "W4g": (np.vstack([W4, W4]), fdt),
        "W4T": (W4.T, bdt), "W3Tg": (np.vstack([W3.T, W3.T]), bdt),
        "W2T": (W2.T, bdt),
        "Wl1b": (W1[:3, :], bdt),
        "nones": (np.full((128, 1), -1.0), bdt),
        "b1e": ((b1 + float(t) * W1[3, :]).reshape(128, 1), np.float32),
        "b2": (b2.reshape(128, 1), np.float32),
        "b3g": (np.concatenate([b3, b3]).reshape(128, 1), np.float32),
        "b4": (b4.reshape(2, 1), np.float32),
    }
    return {("c_" + k): np.ascontiguousarray(v.astype(dt))
            for k, (v, dt) in c.items()}


def kernel(t, y, logp, v, W1, b1, W2, b2, W3, b3, W4, b4):
    global LAST_RESULTS
    del logp  # unused by the reference computation

    n = y.shape[0]
    assert n == N_FULL and n % N_CORES == 0
    ncc = n // N_CORES

    yT = np.ascontiguousarray(np.asarray(y, np.float32).T.astype(_np_dt(FWD_DT)))
    vT = np.ascontiguousarray(np.asarray(v, np.float32).T.astype(_np_dt(BWD_DT)))
    consts = _prep_consts(np.asarray(t).reshape(-1)[0], *[
        np.asarray(a, np.float32) for a in (W1, b1, W2, b2, W3, b3, W4, b4)])

    nc = _build(ncc, F, G)

    in_maps = []
    for c in range(N_CORES):
        sl = slice(c * ncc, (c + 1) * ncc)
        m = dict(consts)
        m["yT"] = np.ascontiguousarray(yT[:, sl])
        m["vT"] = np.ascontiguousarray(vT[:, sl])
        in_maps.append(m)

    trace = os.environ.get("CNF_TRACE", "0") == "1"
    res = run_bass_kernel_spmd(
        nc, in_maps, core_ids=list(range(N_CORES)), trace=trace)
    LAST_RESULTS = res

    dy = np.zeros((n, 3), dtype=np.float32)
    ndiv = np.empty((n, 1), dtype=np.float32)
    for c in range(N_CORES):
        sl = slice(c * ncc, (c + 1) * ncc)
        r = res.results[c]
        dy[sl, 0:2] = r["dyT"].T
        ndiv[sl, 0] = r["ndiv"][0]
    return dy, ndiv


# revision 13
# speedup vs baseline: 1.1141x; 1.1141x over previous
"""Trainium2 Bass kernel for nn_CNF_ODE (dense MLP fwd + Hutchinson divergence).

Contract: kernel(**inputs) takes the FULL unsharded inputs (as produced by
setup_inputs) and returns the full output tuple (dy [N,3], -div [N,1]).

Strategy: pure data parallel over the batch axis across 8 NeuronCores.
Host-side we transpose y/v to feature-major [3, N] (and transpose outputs
back) so the device kernel works entirely in feature-major layout with
contiguous DMAs; the tiny MLP weights are replicated (prepared host-side,
including the transposed weight copies the backward pass needs and the
t-folded first-layer bias).

Device math (feature-major, per batch column):
  z1 = W1[:3]T yT + (b1 + t W1[3]);  h1 = silu(z1); d1 = silu'(z1)
  z2 = W2T h1 + b2;                  h2 = silu(z2); d2 = silu'(z2)
  z3 = W3T h2 + b3;                  h3 = silu(z3); d3 = silu'(z3)
  dy = [W4T h3 + b4; 0]
  u3 = (W4 v[:2]T) * d3
  u2 = (W3 u3) * d2
  p  = (W1[:3]T vT) * d1 * (W2 u2)
  -div = -sum_m p[m]          (column sum via matmul with -1s)

Perf notes:
 - matmuls run in bf16 (fp32 matmul costs 2 HW passes); z accumulation in
   fp32 PSUM; silu/silu' evaluated from the fp32 z.
 - silu and derivative_silu live in different ACT table sets (~2.7us per
   switch), so tiles are processed in groups of G: forward (silu) phase for
   all G tiles, then backward (derivative_silu) phase; z is staged to SBUF
   in fp32 by the DVE so PSUM banks recycle quickly.
"""

import os
from contextlib import ExitStack

import ml_dtypes
import numpy as np

import concourse.bacc as bacc
import concourse.bass as bass
import concourse.mybir as mybir
import concourse.tile as tile
from concourse.bass_utils import run_bass_kernel_spmd
from concourse.tile_rust import add_dep_helper

AF = mybir.ActivationFunctionType
OP = mybir.AluOpType
F32 = mybir.dt.float32
BF16 = mybir.dt.bfloat16

N_CORES = 8
N_FULL = 1048576
NCC = N_FULL // N_CORES  # columns per core
F = 512                  # free-dim tile (one PSUM bank of fp32)
G = 8                    # tiles per table-set phase group

# matmul dtype config ("bf16" or "f32") for forward / backward chains
FWD_DT = BF16
BWD_DT = BF16

_BUILD_CACHE = {}
LAST_RESULTS = None  # BassKernelResults of the most recent run (for test.py)


def _build(ncc=NCC, f=F, g=G):
    key = (ncc, f, g, FWD_DT, BWD_DT)
    if key in _BUILD_CACHE:
        return _BUILD_CACHE[key]

    nc = bacc.Bacc(
        "TRN2",
        target_bir_lowering=False,
        debug=False,
        enable_asserts=False,
        num_devices=N_CORES,
    )
    fdt, bdt = FWD_DT, BWD_DT

    yT_d = nc.dram_tensor("yT", [3, ncc], fdt, kind="ExternalInput").ap()
    vT_d = nc.dram_tensor("vT", [3, ncc], bdt, kind="ExternalInput").ap()
    dyT_d = nc.dram_tensor("dyT", [2, ncc], F32, kind="ExternalOutput").ap()
    nd_d = nc.dram_tensor("ndiv", [1, ncc], F32, kind="ExternalOutput").ap()

    cshape = {
        "Wl1": ([3, 128], fdt), "W2": ([128, 128], fdt),
        "W3": ([128, 64], fdt), "W4g": ([128, 2], fdt),
        "W4T": ([2, 64], bdt), "W3Tg": ([128, 128], bdt),
        "W2T": ([128, 128], bdt), "Wl1b": ([3, 128], bdt),
        "nones": ([128, 1], bdt),
        "b1e": ([128, 1], F32), "b2": ([128, 1], F32),
        "b3g": ([128, 1], F32), "b4": ([2, 1], F32),
    }
    cdram = {k: nc.dram_tensor("c_" + k, s, dt, kind="ExternalInput").ap()
             for k, (s, dt) in cshape.items()}

    ntiles = ncc // f
    assert ntiles * f == ncc and ntiles % g == 0

    def _act(phase_list, *args, **kw):
        ins = nc.scalar.activation(*args, **kw)
        if phase_list is not None:
            phase_list.append(ins)
        return ins

    with tile.TileContext(nc) as tc, ExitStack() as ctx:
        consts = ctx.enter_context(tc.tile_pool(name="consts", bufs=1))
        io = ctx.enter_context(tc.tile_pool(name="io", bufs=3))
        work = ctx.enter_context(tc.tile_pool(name="work", bufs=4))
        zst = ctx.enter_context(tc.tile_pool(name="zst", bufs=g + 2))
        pp = ctx.enter_context(tc.tile_pool(name="pp", bufs=1, space="PSUM"))

        cs = {k: consts.tile(s, dt, name="c" + k, tag="c" + k)
              for k, (s, dt) in cshape.items()}
        for k in cshape:
            nc.sync.dma_start(out=cs[k], in_=cdram[k])

        prev_phase = []
        f2 = 2 * f
        for grp in range(ntiles // g):
            fwd_acts, bwd_acts = [], []
            zs = []
            # ---------- forward phase: silu table set (tile pairs) ----------
            for j in range(0, g, 2):
                it = grp * g + j
                sl = slice(it * f, it * f + f2)
                lo, hi = slice(0, f), slice(f, f2)

                yt = io.tile([3, f2], fdt, tag="yt", bufs=4)
                vt = io.tile([3, f2], bdt, tag="vt", bufs=g + 2)
                nc.sync.dma_start(out=yt, in_=yT_d[:, sl])
                nc.sync.dma_start(out=vt, in_=vT_d[:, sl])

                z1 = pp.tile([128, f2], F32, tag="fw", bufs=2)
                nc.tensor.matmul(out=z1[:, lo], lhsT=cs["Wl1"], rhs=yt[:, lo])
                nc.tensor.matmul(out=z1[:, hi], lhsT=cs["Wl1"], rhs=yt[:, hi])
                h1 = work.tile([128, f2], fdt, tag="h1")
                _act(fwd_acts, h1, z1, AF.Silu, bias=cs["b1e"])
                zs1 = zst.tile([128, f2], F32, tag="zs1", bufs=g // 2 + 2)
                nc.vector.tensor_scalar_add(zs1, z1, cs["b1e"])

                z2 = pp.tile([128, f2], F32, tag="fw", bufs=2)
                nc.tensor.matmul(out=z2[:, lo], lhsT=cs["W2"], rhs=h1[:, lo])
                nc.tensor.matmul(out=z2[:, hi], lhsT=cs["W2"], rhs=h1[:, hi])
                h2 = work.tile([128, f2], fdt, tag="h2")
                _act(fwd_acts, h2, z2, AF.Silu, bias=cs["b2"])
                zs2 = zst.tile([128, f2], F32, tag="zs2", bufs=g // 2 + 2)
                nc.vector.tensor_scalar_add(zs2, z2, cs["b2"])

                # layer 3 row-packed: pair tile A in psum rows 0-63, B in
                # rows 64-127, same columns -> one bank, concurrent matmuls
                # (disjoint PE column groups), and [128, f] ACT/DVE ops.
                z3 = pp.tile([128, f2], F32, tag="fw", bufs=2)
                nc.tensor.matmul(out=z3[0:64, lo], lhsT=cs["W3"], rhs=h2[:, lo],
                                 tile_position=(0, 0))
                nc.tensor.matmul(out=z3[64:128, lo], lhsT=cs["W3"], rhs=h2[:, hi],
                                 tile_position=(0, 64))
                h3 = work.tile([128, f], fdt, tag="h3")
                _act(fwd_acts, h3, z3[:, lo], AF.Silu, bias=cs["b3g"])
                zs3 = zst.tile([128, f], F32, tag="zs3", bufs=g // 2 + 2)
                nc.vector.tensor_scalar_add(zs3, z3[:, lo], cs["b3g"])

                z4 = pp.tile([128, f2], F32, tag="fw", bufs=2)
                nc.tensor.matmul(out=z4[0:2, lo], lhsT=cs["W4g"][0:64, :],
                                 rhs=h3[0:64, :], tile_position=(0, 0))
                nc.tensor.matmul(out=z4[0:2, hi], lhsT=cs["W4g"][64:128, :],
                                 rhs=h3[64:128, :], tile_position=(64, 0))
                dy_t = io.tile([2, f2], F32, tag="dy", bufs=3)
                nc.vector.tensor_scalar_add(dy_t, z4[0:2, :], cs["b4"])
                nc.sync.dma_start(out=dyT_d[:, sl], in_=dy_t)

                zs.append((vt, zs1, zs2, zs3, sl))

            # ------- backward phase: derivative_silu table set (pairs) -------
            for pj in range(g // 2):
                vt, zs1, zs2, zs3, sl = zs[pj]
                lo, hi = slice(0, f), slice(f, f2)

                d3 = work.tile([128, f], bdt, tag="d3")
                _act(bwd_acts, d3, zs3, AF.Derivative_silu)
                wps = pp.tile([128, f2], F32, tag="bw", bufs=2)
                nc.tensor.matmul(out=wps[0:64, lo], lhsT=cs["W4T"],
                                 rhs=vt[0:2, lo], tile_position=(0, 0))
                nc.tensor.matmul(out=wps[64:128, lo], lhsT=cs["W4T"],
                                 rhs=vt[0:2, hi], tile_position=(0, 64))
                u3 = work.tile([128, f], bdt, tag="u3")
                nc.vector.tensor_mul(u3, d3, wps[:, lo])

                u2ps = pp.tile([128, f2], F32, tag="bw", bufs=2)
                nc.tensor.matmul(out=u2ps[:, lo], lhsT=cs["W3Tg"][0:64, :],
                                 rhs=u3[0:64, :], tile_position=(0, 0))
                nc.tensor.matmul(out=u2ps[:, hi], lhsT=cs["W3Tg"][64:128, :],
                                 rhs=u3[64:128, :], tile_position=(64, 0))
                d2 = work.tile([128, f2], bdt, tag="d2")
                _act(bwd_acts, d2, zs2, AF.Derivative_silu)
                u2 = work.tile([128, f2], bdt, tag="u2")
                nc.vector.tensor_mul(u2, d2, u2ps)

                u1ps = pp.tile([128, f2], F32, tag="bw", bufs=2)
                nc.tensor.matmul(out=u1ps[:, lo], lhsT=cs["W2T"], rhs=u2[:, lo])
                nc.tensor.matmul(out=u1ps[:, hi], lhsT=cs["W2T"], rhs=u2[:, hi])
                d1 = work.tile([128, f2], bdt, tag="d1")
                _act(bwd_acts, d1, zs1, AF.Derivative_silu)

                vps = pp.tile([128, f2], F32, tag="bw", bufs=2)
                nc.tensor.matmul(out=vps[:, lo], lhsT=cs["Wl1b"], rhs=vt[:, lo])
                nc.tensor.matmul(out=vps[:, hi], lhsT=cs["Wl1b"], rhs=vt[:, hi])
                vd = work.tile([128, f2], bdt, tag="vd")
                nc.vector.tensor_mul(vd, d1, vps)
                p = work.tile([128, f2], bdt, tag="p")
                nc.vector.tensor_mul(p, vd, u1ps)

                dv = pp.tile([128, f2], F32, tag="bw", bufs=2)
                nc.tensor.matmul(out=dv[0:1, lo], lhsT=cs["nones"], rhs=p[:, lo])
                nc.tensor.matmul(out=dv[0:1, hi], lhsT=cs["nones"], rhs=p[:, hi])
                nd_t = io.tile([1, f2], F32, tag="nd", bufs=3)
                _act(None, nd_t, dv[0:1, :], AF.Copy)
                nc.sync.dma_start(out=nd_d[:, sl], in_=nd_t)

            # table-set phase purity (ACT-stream ordering only, no sems)
            for a in fwd_acts:
                for b in prev_phase:
                    add_dep_helper(a.ins, b.ins, sync=False, reason="actset")
            for b in bwd_acts:
                for a in fwd_acts:
                    add_dep_helper(b.ins, a.ins, sync=False, reason="actset")
            prev_phase = bwd_acts

    nc.compile()
    _BUILD_CACHE[key] = nc
    return nc


def _np_dt(dt):
    return ml_dtypes.bfloat16 if dt == BF16 else np.float32


def _prep_consts(t, W1, b1, W2, b2, W3, b3, W4, b4):
    fdt, bdt = _np_dt(FWD_DT), _np_dt(BWD_DT)
    c = {
        "Wl1": (W1[:3, :], fdt), "W2": (W2, fdt), "W3": (W3, fdt),
        "W4g": (np.vstack([W4, W4]), fdt),
        "W4T": (W4.T, bdt), "W3Tg": (np.vstack([W3.T, W3.T]), bdt),
        "W2T": (W2.T, bdt),
        "Wl1b": (W1[:3, :], bdt),
        "nones": (np.full((128, 1), -1.0), bdt),
        "b1e": ((b1 + float(t) * W1[3, :]).reshape(128, 1), np.float32),
        "b2": (b2.reshape(128, 1), np.float32),
        "b3g": (np.concatenate([b3, b3]).reshape(128, 1), np.float32),
        "b4": (b4.reshape(2, 1), np.float32),
    }
    return {("c_" + k): np.ascontiguousarray(v.astype(dt))
            for k, (v, dt) in c.items()}


def kernel(t, y, logp, v, W1, b1, W2, b2, W3, b3, W4, b4):
    global LAST_RESULTS
    del logp  # unused by the reference computation

    n = y.shape[0]
    assert n == N_FULL and n % N_CORES == 0
    ncc = n // N_CORES

    yT = np.ascontiguousarray(np.asarray(y, np.float32).T.astype(_np_dt(FWD_DT)))
    vT = np.ascontiguousarray(np.asarray(v, np.float32).T.astype(_np_dt(BWD_DT)))
    consts = _prep_consts(np.asarray(t).reshape(-1)[0], *[
        np.asarray(a, np.float32) for a in (W1, b1, W2, b2, W3, b3, W4, b4)])

    nc = _build(ncc, F, G)

    in_maps = []
    for c in range(N_CORES):
        sl = slice(c * ncc, (c + 1) * ncc)
        m = dict(consts)
        m["yT"] = np.ascontiguousarray(yT[:, sl])
        m["vT"] = np.ascontiguousarray(vT[:, sl])
        in_maps.append(m)

    trace = os.environ.get("CNF_TRACE", "0") == "1"
    res = run_bass_kernel_spmd(
        nc, in_maps, core_ids=list(range(N_CORES)), trace=trace)
    LAST_RESULTS = res

    dy = np.zeros((n, 3), dtype=np.float32)
    ndiv = np.empty((n, 1), dtype=np.float32)
    for c in range(N_CORES):
        sl = slice(c * ncc, (c + 1) * ncc)
        r = res.results[c]
        dy[sl, 0:2] = r["dyT"].T
        ndiv[sl, 0] = r["ndiv"][0]
    return dy, ndiv


# revision 14
# speedup vs baseline: 1.1694x; 1.0496x over previous
"""Trainium2 Bass kernel for nn_CNF_ODE (dense MLP fwd + Hutchinson divergence).

Contract: kernel(**inputs) takes the FULL unsharded inputs (as produced by
setup_inputs) and returns the full output tuple (dy [N,3], -div [N,1]).

Strategy: pure data parallel over the batch axis across 8 NeuronCores.
Host-side we transpose y/v to feature-major [3, N] (and transpose outputs
back) so the device kernel works entirely in feature-major layout with
contiguous DMAs; the tiny MLP weights are replicated (prepared host-side,
including the transposed weight copies the backward pass needs and the
t-folded first-layer bias).

Device math (feature-major, per batch column):
  z1 = W1[:3]T yT + (b1 + t W1[3]);  h1 = silu(z1); d1 = silu'(z1)
  z2 = W2T h1 + b2;                  h2 = silu(z2); d2 = silu'(z2)
  z3 = W3T h2 + b3;                  h3 = silu(z3); d3 = silu'(z3)
  dy = [W4T h3 + b4; 0]
  u3 = (W4 v[:2]T) * d3
  u2 = (W3 u3) * d2
  p  = (W1[:3]T vT) * d1 * (W2 u2)
  -div = -sum_m p[m]          (column sum via matmul with -1s)

Perf notes:
 - matmuls run in bf16 (fp32 matmul costs 2 HW passes); z accumulation in
   fp32 PSUM; silu/silu' evaluated from the fp32 z.
 - silu and derivative_silu live in different ACT table sets (~2.7us per
   switch), so tiles are processed in groups of G: forward (silu) phase for
   all G tiles, then backward (derivative_silu) phase; z is staged to SBUF
   in fp32 by the DVE so PSUM banks recycle quickly.
"""

import os
from contextlib import ExitStack

import ml_dtypes
import numpy as np

import concourse.bacc as bacc
import concourse.bass as bass
import concourse.mybir as mybir
import concourse.tile as tile
from concourse.bass_utils import run_bass_kernel_spmd
from concourse.tile_rust import add_dep_helper

AF = mybir.ActivationFunctionType
OP = mybir.AluOpType
F32 = mybir.dt.float32
BF16 = mybir.dt.bfloat16

N_CORES = 8
N_FULL = 1048576
NCC = N_FULL // N_CORES  # columns per core
F = 512                  # free-dim tile (one PSUM bank of fp32)
G = 8                    # tiles per table-set phase group

# matmul dtype config ("bf16" or "f32") for forward / backward chains
FWD_DT = BF16
BWD_DT = BF16

_BUILD_CACHE = {}
LAST_RESULTS = None  # BassKernelResults of the most recent run (for test.py)


def _build(ncc=NCC, f=F, g=G):
    key = (ncc, f, g, FWD_DT, BWD_DT)
    if key in _BUILD_CACHE:
        return _BUILD_CACHE[key]

    nc = bacc.Bacc(
        "TRN2",
        target_bir_lowering=False,
        debug=False,
        enable_asserts=False,
        num_devices=N_CORES,
    )
    fdt, bdt = FWD_DT, BWD_DT

    yT_d = nc.dram_tensor("yT", [3, ncc], fdt, kind="ExternalInput").ap()
    vT_d = nc.dram_tensor("vT", [3, ncc], bdt, kind="ExternalInput").ap()
    dyT_d = nc.dram_tensor("dyT", [2, ncc], F32, kind="ExternalOutput").ap()
    nd_d = nc.dram_tensor("ndiv", [1, ncc], F32, kind="ExternalOutput").ap()

    cshape = {
        "Wl1g": ([128, 128], fdt), "W2": ([128, 128], fdt),
        "W3": ([128, 64], fdt), "W4g": ([128, 2], fdt),
        "W4Tg2": ([128, 64], bdt), "W3Tg": ([128, 128], bdt),
        "W2T": ([128, 128], bdt), "Wl1bg": ([128, 128], bdt),
        "nones": ([128, 1], bdt),
        "b1e": ([128, 1], F32), "b2": ([128, 1], F32),
        "b3g": ([128, 1], F32), "b4": ([2, 1], F32),
    }
    cdram = {k: nc.dram_tensor("c_" + k, s, dt, kind="ExternalInput").ap()
             for k, (s, dt) in cshape.items()}

    ntiles = ncc // f
    assert ntiles * f == ncc and ntiles % g == 0

    def _act(phase_list, *args, **kw):
        ins = nc.scalar.activation(*args, **kw)
        if phase_list is not None:
            phase_list.append(ins)
        return ins

    with tile.TileContext(nc) as tc, ExitStack() as ctx:
        consts = ctx.enter_context(tc.tile_pool(name="consts", bufs=1))
        io = ctx.enter_context(tc.tile_pool(name="io", bufs=3))
        work = ctx.enter_context(tc.tile_pool(name="work", bufs=4))
        zst = ctx.enter_context(tc.tile_pool(name="zst", bufs=g + 2))
        pp = ctx.enter_context(tc.tile_pool(name="pp", bufs=1, space="PSUM"))

        cs = {k: consts.tile(s, dt, name="c" + k, tag="c" + k)
              for k, (s, dt) in cshape.items()}
        for k in cshape:
            nc.sync.dma_start(out=cs[k], in_=cdram[k])

        prev_phase = []
        f2 = 2 * f
        for grp in range(ntiles // g):
            fwd_acts, bwd_acts = [], []
            zs = []
            # ---------- forward phase: silu table set (tile pairs) ----------
            for j in range(0, g, 2):
                it = grp * g + j
                sl = slice(it * f, it * f + f2)
                lo, hi = slice(0, f), slice(f, f2)

                slo = slice(it * f, it * f + f)
                shi = slice(it * f + f, it * f + f2)
                yt = io.tile([128, f], fdt, tag="yt", bufs=4)
                vt = io.tile([128, f], bdt, tag="vt", bufs=g + 2)
                nc.sync.dma_start(out=yt[0:3, :], in_=yT_d[:, slo])
                nc.sync.dma_start(out=yt[32:35, :], in_=yT_d[:, shi])
                nc.sync.dma_start(out=vt[0:3, :], in_=vT_d[:, slo])
                nc.sync.dma_start(out=vt[32:35, :], in_=vT_d[:, shi])

                z1 = pp.tile([128, f2], F32, tag="fw", bufs=2)
                nc.tensor.matmul(out=z1[:, lo], lhsT=cs["Wl1g"][0:3, :],
                                 rhs=yt[0:3, :], tile_position=(0, 0))
                nc.tensor.matmul(out=z1[:, hi], lhsT=cs["Wl1g"][32:35, :],
                                 rhs=yt[32:35, :], tile_position=(32, 0))
                h1 = work.tile([128, f2], fdt, tag="h1")
                _act(fwd_acts, h1, z1, AF.Silu, bias=cs["b1e"])
                zs1 = zst.tile([128, f2], F32, tag="zs1", bufs=g // 2 + 2)
                nc.vector.tensor_scalar_add(zs1, z1, cs["b1e"])

                z2 = pp.tile([128, f2], F32, tag="fw", bufs=2)
                nc.tensor.matmul(out=z2[:, lo], lhsT=cs["W2"], rhs=h1[:, lo])
                nc.tensor.matmul(out=z2[:, hi], lhsT=cs["W2"], rhs=h1[:, hi])
                h2 = work.tile([128, f2], fdt, tag="h2")
                _act(fwd_acts, h2, z2, AF.Silu, bias=cs["b2"])
                zs2 = zst.tile([128, f2], F32, tag="zs2", bufs=g // 2 + 2)
                nc.vector.tensor_scalar_add(zs2, z2, cs["b2"])

                # layer 3 row-packed: pair tile A in psum rows 0-63, B in
                # rows 64-127, same columns -> one bank, concurrent matmuls
                # (disjoint PE column groups), and [128, f] ACT/DVE ops.
                z3 = pp.tile([128, f2], F32, tag="fw", bufs=2)
                nc.tensor.matmul(out=z3[0:64, lo], lhsT=cs["W3"], rhs=h2[:, lo],
                                 tile_position=(0, 0))
                nc.tensor.matmul(out=z3[64:128, lo], lhsT=cs["W3"], rhs=h2[:, hi],
                                 tile_position=(0, 64))
                h3 = work.tile([128, f], fdt, tag="h3")
                _act(fwd_acts, h3, z3[:, lo], AF.Silu, bias=cs["b3g"])
                zs3 = zst.tile([128, f], F32, tag="zs3", bufs=g // 2 + 2)
                nc.vector.tensor_scalar_add(zs3, z3[:, lo], cs["b3g"])

                z4 = pp.tile([128, f2], F32, tag="fw", bufs=2)
                nc.tensor.matmul(out=z4[0:2, lo], lhsT=cs["W4g"][0:64, :],
                                 rhs=h3[0:64, :], tile_position=(0, 0))
                nc.tensor.matmul(out=z4[0:2, hi], lhsT=cs["W4g"][64:128, :],
                                 rhs=h3[64:128, :], tile_position=(64, 0))
                dy_t = io.tile([2, f2], F32, tag="dy", bufs=3)
                nc.vector.tensor_scalar_add(dy_t, z4[0:2, :], cs["b4"])
                nc.sync.dma_start(out=dyT_d[:, sl], in_=dy_t)

                zs.append((vt, zs1, zs2, zs3, sl))

            # ------- backward phase: derivative_silu table set (pairs) -------
            for pj in range(g // 2):
                vt, zs1, zs2, zs3, sl = zs[pj]
                lo, hi = slice(0, f), slice(f, f2)

                d3 = work.tile([128, f], bdt, tag="d3")
                _act(bwd_acts, d3, zs3, AF.Derivative_silu)
                wps = pp.tile([128, f2], F32, tag="bw", bufs=2)
                nc.tensor.matmul(out=wps[0:64, lo], lhsT=cs["W4Tg2"][0:2, :],
                                 rhs=vt[0:2, :], tile_position=(0, 0))
                nc.tensor.matmul(out=wps[64:128, lo], lhsT=cs["W4Tg2"][32:34, :],
                                 rhs=vt[32:34, :], tile_position=(32, 64))
                u3 = work.tile([128, f], bdt, tag="u3")
                nc.vector.tensor_mul(u3, d3, wps[:, lo])

                u2ps = pp.tile([128, f2], F32, tag="bw", bufs=2)
                nc.tensor.matmul(out=u2ps[:, lo], lhsT=cs["W3Tg"][0:64, :],
                                 rhs=u3[0:64, :], tile_position=(0, 0))
                nc.tensor.matmul(out=u2ps[:, hi], lhsT=cs["W3Tg"][64:128, :],
                                 rhs=u3[64:128, :], tile_position=(64, 0))
                d2 = work.tile([128, f2], bdt, tag="d2")
                _act(bwd_acts, d2, zs2, AF.Derivative_silu)
                u2 = work.tile([128, f2], bdt, tag="u2")
                nc.vector.tensor_mul(u2, d2, u2ps)

                u1ps = pp.tile([128, f2], F32, tag="bw", bufs=2)
                nc.tensor.matmul(out=u1ps[:, lo], lhsT=cs["W2T"], rhs=u2[:, lo])
                nc.tensor.matmul(out=u1ps[:, hi], lhsT=cs["W2T"], rhs=u2[:, hi])
                d1 = work.tile([128, f2], bdt, tag="d1")
                _act(bwd_acts, d1, zs1, AF.Derivative_silu)

                vps = pp.tile([128, f2], F32, tag="bw", bufs=2)
                nc.tensor.matmul(out=vps[:, lo], lhsT=cs["Wl1bg"][0:3, :],
                                 rhs=vt[0:3, :], tile_position=(0, 0))
                nc.tensor.matmul(out=vps[:, hi], lhsT=cs["Wl1bg"][32:35, :],
                                 rhs=vt[32:35, :], tile_position=(32, 0))
                vd = work.tile([128, f2], bdt, tag="vd")
                nc.vector.tensor_mul(vd, d1, vps)
                p = work.tile([128, f2], bdt, tag="p")
                nc.vector.tensor_mul(p, vd, u1ps)

                dv = pp.tile([128, f2], F32, tag="bw", bufs=2)
                nc.tensor.matmul(out=dv[0:1, lo], lhsT=cs["nones"], rhs=p[:, lo])
                nc.tensor.matmul(out=dv[0:1, hi], lhsT=cs["nones"], rhs=p[:, hi])
                nd_t = io.tile([1, f2], F32, tag="nd", bufs=3)
                _act(None, nd_t, dv[0:1, :], AF.Copy)
                nc.sync.dma_start(out=nd_d[:, sl], in_=nd_t)

            # table-set phase purity (ACT-stream ordering only, no sems)
            for a in fwd_acts:
                for b in prev_phase:
                    add_dep_helper(a.ins, b.ins, sync=False, reason="actset")
            for b in bwd_acts:
                for a in fwd_acts:
                    add_dep_helper(b.ins, a.ins, sync=False, reason="actset")
            prev_phase = bwd_acts

    nc.compile()
    _BUILD_CACHE[key] = nc
    return nc


def _np_dt(dt):
    return ml_dtypes.bfloat16 if dt == BF16 else np.float32


def _dup32(w, m):
    out = np.zeros((128, m), np.float32)
    out[0:w.shape[0], :] = w
    out[32:32 + w.shape[0], :] = w
    return out


def _prep_consts(t, W1, b1, W2, b2, W3, b3, W4, b4):
    fdt, bdt = _np_dt(FWD_DT), _np_dt(BWD_DT)
    c = {
        "Wl1g": (_dup32(W1[:3, :], 128), fdt), "W2": (W2, fdt),
        "W3": (W3, fdt),
        "W4g": (np.vstack([W4, W4]), fdt),
        "W4Tg2": (_dup32(W4.T, 64), bdt),
        "W3Tg": (np.vstack([W3.T, W3.T]), bdt),
        "W2T": (W2.T, bdt),
        "Wl1bg": (_dup32(W1[:3, :], 128), bdt),
        "nones": (np.full((128, 1), -1.0), bdt),
        "b1e": ((b1 + float(t) * W1[3, :]).reshape(128, 1), np.float32),
        "b2": (b2.reshape(128, 1), np.float32),
        "b3g": (np.concatenate([b3, b3]).reshape(128, 1), np.float32),
        "b4": (b4.reshape(2, 1), np.float32),
    }
    return {("c_" + k): np.ascontiguousarray(v.astype(dt))
            for k, (v, dt) in c.items()}


def kernel(t, y, logp, v, W1, b1, W2, b2, W3, b3, W4, b4):
    global LAST_RESULTS
    del logp  # unused by the reference computation

    n = y.shape[0]
    assert n == N_FULL and n % N_CORES == 0
    ncc = n // N_CORES

    yT = np.ascontiguousarray(np.asarray(y, np.float32).T.astype(_np_dt(FWD_DT)))
    vT = np.ascontiguousarray(np.asarray(v, np.float32).T.astype(_np_dt(BWD_DT)))
    consts = _prep_consts(np.asarray(t).reshape(-1)[0], *[
        np.asarray(a, np.float32) for a in (W1, b1, W2, b2, W3, b3, W4, b4)])

    nc = _build(ncc, F, G)

    in_maps = []
    for c in range(N_CORES):
        sl = slice(c * ncc, (c + 1) * ncc)
        m = dict(consts)
        m["yT"] = np.ascontiguousarray(yT[:, sl])
        m["vT"] = np.ascontiguousarray(vT[:, sl])
        in_maps.append(m)

    trace = os.environ.get("CNF_TRACE", "0") == "1"
    res = run_bass_kernel_spmd(
        nc, in_maps, core_ids=list(range(N_CORES)), trace=trace)
    LAST_RESULTS = res

    dy = np.zeros((n, 3), dtype=np.float32)
    ndiv = np.empty((n, 1), dtype=np.float32)
    for c in range(N_CORES):
        sl = slice(c * ncc, (c + 1) * ncc)
        r = res.results[c]
        dy[sl, 0:2] = r["dyT"].T
        ndiv[sl, 0] = r["ndiv"][0]
    return dy, ndiv


# revision 15
# speedup vs baseline: 1.1907x; 1.0182x over previous
"""Trainium2 Bass kernel for nn_CNF_ODE (dense MLP fwd + Hutchinson divergence).

Contract: kernel(**inputs) takes the FULL unsharded inputs (as produced by
setup_inputs) and returns the full output tuple (dy [N,3], -div [N,1]).

Strategy: pure data parallel over the batch axis across 8 NeuronCores.
Host-side we transpose y/v to feature-major [3, N] (and transpose outputs
back) so the device kernel works entirely in feature-major layout with
contiguous DMAs; the tiny MLP weights are replicated (prepared host-side,
including the transposed weight copies the backward pass needs and the
t-folded first-layer bias).

Device math (feature-major, per batch column):
  z1 = W1[:3]T yT + (b1 + t W1[3]);  h1 = silu(z1); d1 = silu'(z1)
  z2 = W2T h1 + b2;                  h2 = silu(z2); d2 = silu'(z2)
  z3 = W3T h2 + b3;                  h3 = silu(z3); d3 = silu'(z3)
  dy = [W4T h3 + b4; 0]
  u3 = (W4 v[:2]T) * d3
  u2 = (W3 u3) * d2
  p  = (W1[:3]T vT) * d1 * (W2 u2)
  -div = -sum_m p[m]          (column sum via matmul with -1s)

Perf notes:
 - matmuls run in bf16 (fp32 matmul costs 2 HW passes); z accumulation in
   fp32 PSUM; silu/silu' evaluated from the fp32 z.
 - silu and derivative_silu live in different ACT table sets (~2.7us per
   switch), so tiles are processed in groups of G: forward (silu) phase for
   all G tiles, then backward (derivative_silu) phase; z is staged to SBUF
   in fp32 by the DVE so PSUM banks recycle quickly.
"""

import os
from contextlib import ExitStack

import ml_dtypes
import numpy as np

import concourse.bacc as bacc
import concourse.bass as bass
import concourse.mybir as mybir
import concourse.tile as tile
from concourse.bass_utils import run_bass_kernel_spmd
from concourse.tile_rust import add_dep_helper

AF = mybir.ActivationFunctionType
OP = mybir.AluOpType
F32 = mybir.dt.float32
BF16 = mybir.dt.bfloat16

N_CORES = 8
N_FULL = 1048576
NCC = N_FULL // N_CORES  # columns per core
F = 512                  # free-dim tile (one PSUM bank of fp32)
G = 8                    # tiles per table-set phase group

# matmul dtype config ("bf16" or "f32") for forward / backward chains
FWD_DT = BF16
BWD_DT = BF16

_BUILD_CACHE = {}
LAST_RESULTS = None  # BassKernelResults of the most recent run (for test.py)


def _build(ncc=NCC, f=F, g=G):
    key = (ncc, f, g, FWD_DT, BWD_DT)
    if key in _BUILD_CACHE:
        return _BUILD_CACHE[key]

    nc = bacc.Bacc(
        "TRN2",
        target_bir_lowering=False,
        debug=False,
        enable_asserts=False,
        num_devices=N_CORES,
    )
    fdt, bdt = FWD_DT, BWD_DT

    yT_d = nc.dram_tensor("yT", [3, ncc], fdt, kind="ExternalInput").ap()
    vT_d = nc.dram_tensor("vT", [3, ncc], bdt, kind="ExternalInput").ap()
    dyT_d = nc.dram_tensor("dyT", [2, ncc], F32, kind="ExternalOutput").ap()
    nd_d = nc.dram_tensor("ndiv", [1, ncc], F32, kind="ExternalOutput").ap()

    cshape = {
        "Wl1g": ([128, 128], fdt), "W2": ([128, 128], fdt),
        "W3": ([128, 64], fdt), "W4g": ([128, 2], fdt),
        "W4Tg2": ([128, 64], bdt), "W3Tg": ([128, 128], bdt),
        "W2T": ([128, 128], bdt), "Wl1bg": ([128, 128], bdt),
        "nones": ([128, 1], bdt),
        "b1e": ([128, 1], F32), "b2": ([128, 1], F32),
        "b3g": ([128, 1], F32), "b4": ([2, 1], F32),
    }
    cdram = {k: nc.dram_tensor("c_" + k, s, dt, kind="ExternalInput").ap()
             for k, (s, dt) in cshape.items()}

    ntiles = ncc // f
    assert ntiles * f == ncc and ntiles % g == 0

    def _act(phase_list, *args, **kw):
        ins = nc.scalar.activation(*args, **kw)
        if phase_list is not None:
            phase_list.append(ins)
        return ins

    with tile.TileContext(nc) as tc, ExitStack() as ctx:
        consts = ctx.enter_context(tc.tile_pool(name="consts", bufs=1))
        io = ctx.enter_context(tc.tile_pool(name="io", bufs=3))
        work = ctx.enter_context(tc.tile_pool(name="work", bufs=4))
        zst = ctx.enter_context(tc.tile_pool(name="zst", bufs=g + 2))
        pp = ctx.enter_context(tc.tile_pool(name="pp", bufs=1, space="PSUM"))

        cs = {k: consts.tile(s, dt, name="c" + k, tag="c" + k)
              for k, (s, dt) in cshape.items()}
        for k in cshape:
            nc.sync.dma_start(out=cs[k], in_=cdram[k])

        prev_phase = []
        f2 = 2 * f
        f4 = 4 * f
        for grp in range(ntiles // g):
            fwd_acts, bwd_acts = [], []
            zs = []
            zq = [None]
            # ---------- forward phase: silu table set (tile pairs) ----------
            for j in range(0, g, 2):
                it = grp * g + j
                sl = slice(it * f, it * f + f2)
                lo, hi = slice(0, f), slice(f, f2)

                slo = slice(it * f, it * f + f)
                shi = slice(it * f + f, it * f + f2)
                yt = io.tile([128, f], fdt, tag="yt", bufs=4)
                vt = io.tile([128, f], bdt, tag="vt", bufs=g + 2)
                nc.sync.dma_start(out=yt[0:3, :], in_=yT_d[:, slo])
                nc.sync.dma_start(out=yt[32:35, :], in_=yT_d[:, shi])
                nc.sync.dma_start(out=vt[0:3, :], in_=vT_d[:, slo])
                nc.sync.dma_start(out=vt[32:35, :], in_=vT_d[:, shi])

                if (j // 2) % 2 == 0:
                    zq1 = zst.tile([128, f4], F32, tag="zs1", bufs=3)
                    zq2 = zst.tile([128, f4], F32, tag="zs2", bufs=3)
                    zq3 = zst.tile([128, f2], F32, tag="zs3", bufs=3)
                    zq[0] = (zq1, zq2, zq3)
                zq1, zq2, zq3 = zq[0]
                qh = slice(0, f2) if (j // 2) % 2 == 0 else slice(f2, f4)
                qh3 = slice(0, f) if (j // 2) % 2 == 0 else slice(f, f2)

                z1 = pp.tile([128, f2], F32, tag="fw", bufs=2)
                nc.tensor.matmul(out=z1[:, lo], lhsT=cs["Wl1g"][0:3, :],
                                 rhs=yt[0:3, :], tile_position=(0, 0))
                nc.tensor.matmul(out=z1[:, hi], lhsT=cs["Wl1g"][32:35, :],
                                 rhs=yt[32:35, :], tile_position=(32, 0))
                h1 = work.tile([128, f2], fdt, tag="h1")
                _act(fwd_acts, h1, z1, AF.Silu, bias=cs["b1e"])
                nc.vector.tensor_scalar_add(zq1[:, qh], z1, cs["b1e"])

                z2 = pp.tile([128, f2], F32, tag="fw", bufs=2)
                nc.tensor.matmul(out=z2[:, lo], lhsT=cs["W2"], rhs=h1[:, lo])
                nc.tensor.matmul(out=z2[:, hi], lhsT=cs["W2"], rhs=h1[:, hi])
                h2 = work.tile([128, f2], fdt, tag="h2")
                _act(fwd_acts, h2, z2, AF.Silu, bias=cs["b2"])
                nc.vector.tensor_scalar_add(zq2[:, qh], z2, cs["b2"])

                # layer 3 row-packed: pair tile A in psum rows 0-63, B in
                # rows 64-127, same columns -> one bank, concurrent matmuls
                # (disjoint PE column groups), and [128, f] ACT/DVE ops.
                z3 = pp.tile([128, f2], F32, tag="fw", bufs=2)
                nc.tensor.matmul(out=z3[0:64, lo], lhsT=cs["W3"], rhs=h2[:, lo],
                                 tile_position=(0, 0))
                nc.tensor.matmul(out=z3[64:128, lo], lhsT=cs["W3"], rhs=h2[:, hi],
                                 tile_position=(0, 64))
                h3 = work.tile([128, f], fdt, tag="h3")
                _act(fwd_acts, h3, z3[:, lo], AF.Silu, bias=cs["b3g"])
                nc.vector.tensor_scalar_add(zq3[:, qh3], z3[:, lo], cs["b3g"])

                z4 = pp.tile([128, f2], F32, tag="fw", bufs=2)
                nc.tensor.matmul(out=z4[0:2, lo], lhsT=cs["W4g"][0:64, :],
                                 rhs=h3[0:64, :], tile_position=(0, 0))
                nc.tensor.matmul(out=z4[0:2, hi], lhsT=cs["W4g"][64:128, :],
                                 rhs=h3[64:128, :], tile_position=(64, 0))
                dy_t = io.tile([2, f2], F32, tag="dy", bufs=3)
                nc.vector.tensor_scalar_add(dy_t, z4[0:2, :], cs["b4"])
                nc.sync.dma_start(out=dyT_d[:, sl], in_=dy_t)

                zs.append((vt, zq[0], (j // 2) % 2, sl))

            # ------- backward phase: derivative_silu table set (pairs) -------
            dq = [None]
            for pj in range(g // 2):
                vt, (zq1, zq2, zq3), qi, sl = zs[pj]
                lo, hi = slice(0, f), slice(f, f2)

                if qi == 0:
                    dq1 = work.tile([128, f4], F32, tag="dq1", bufs=2)
                    _act(bwd_acts, dq1, zq1, AF.Derivative_silu)
                    dq2 = work.tile([128, f4], F32, tag="dq2", bufs=2)
                    _act(bwd_acts, dq2, zq2, AF.Derivative_silu)
                    dq3 = work.tile([128, f2], F32, tag="dq3", bufs=2)
                    _act(bwd_acts, dq3, zq3, AF.Derivative_silu)
                    dq[0] = (dq1, dq2, dq3)
                dq1, dq2, dq3 = dq[0]
                qh = slice(0, f2) if qi == 0 else slice(f2, f4)
                qh3 = slice(0, f) if qi == 0 else slice(f, f2)
                d3 = dq3[:, qh3]
                d2 = dq2[:, qh]
                d1 = dq1[:, qh]
                wps = pp.tile([128, f2], F32, tag="bw", bufs=2)
                nc.tensor.matmul(out=wps[0:64, lo], lhsT=cs["W4Tg2"][0:2, :],
                                 rhs=vt[0:2, :], tile_position=(0, 0))
                nc.tensor.matmul(out=wps[64:128, lo], lhsT=cs["W4Tg2"][32:34, :],
                                 rhs=vt[32:34, :], tile_position=(32, 64))
                u3 = work.tile([128, f], bdt, tag="u3")
                nc.vector.tensor_mul(u3, d3, wps[:, lo])

                u2ps = pp.tile([128, f2], F32, tag="bw", bufs=2)
                nc.tensor.matmul(out=u2ps[:, lo], lhsT=cs["W3Tg"][0:64, :],
                                 rhs=u3[0:64, :], tile_position=(0, 0))
                nc.tensor.matmul(out=u2ps[:, hi], lhsT=cs["W3Tg"][64:128, :],
                                 rhs=u3[64:128, :], tile_position=(64, 0))
                u2 = work.tile([128, f2], bdt, tag="u2")
                nc.vector.tensor_mul(u2, d2, u2ps)

                u1ps = pp.tile([128, f2], F32, tag="bw", bufs=2)
                nc.tensor.matmul(out=u1ps[:, lo], lhsT=cs["W2T"], rhs=u2[:, lo])
                nc.tensor.matmul(out=u1ps[:, hi], lhsT=cs["W2T"], rhs=u2[:, hi])

                vps = pp.tile([128, f2], F32, tag="bw", bufs=2)
                nc.tensor.matmul(out=vps[:, lo], lhsT=cs["Wl1bg"][0:3, :],
                                 rhs=vt[0:3, :], tile_position=(0, 0))
                nc.tensor.matmul(out=vps[:, hi], lhsT=cs["Wl1bg"][32:35, :],
                                 rhs=vt[32:35, :], tile_position=(32, 0))
                vd = work.tile([128, f2], bdt, tag="vd")
                nc.vector.tensor_mul(vd, d1, vps)
                p = work.tile([128, f2], bdt, tag="p")
                nc.vector.tensor_mul(p, vd, u1ps)

                dv = pp.tile([128, f2], F32, tag="bw", bufs=2)
                nc.tensor.matmul(out=dv[0:1, lo], lhsT=cs["nones"], rhs=p[:, lo])
                nc.tensor.matmul(out=dv[0:1, hi], lhsT=cs["nones"], rhs=p[:, hi])
                nd_t = io.tile([1, f2], F32, tag="nd", bufs=3)
                _act(None, nd_t, dv[0:1, :], AF.Copy)
                nc.sync.dma_start(out=nd_d[:, sl], in_=nd_t)

            # table-set phase purity (ACT-stream ordering only, no sems)
            for a in fwd_acts:
                for b in prev_phase:
                    add_dep_helper(a.ins, b.ins, sync=False, reason="actset")
            for b in bwd_acts:
                for a in fwd_acts:
                    add_dep_helper(b.ins, a.ins, sync=False, reason="actset")
            prev_phase = bwd_acts

    nc.compile()
    _BUILD_CACHE[key] = nc
    return nc


def _np_dt(dt):
    return ml_dtypes.bfloat16 if dt == BF16 else np.float32


def _dup32(w, m):
    out = np.zeros((128, m), np.float32)
    out[0:w.shape[0], :] = w
    out[32:32 + w.shape[0], :] = w
    return out


def _prep_consts(t, W1, b1, W2, b2, W3, b3, W4, b4):
    fdt, bdt = _np_dt(FWD_DT), _np_dt(BWD_DT)
    c = {
        "Wl1g": (_dup32(W1[:3, :], 128), fdt), "W2": (W2, fdt),
        "W3": (W3, fdt),
        "W4g": (np.vstack([W4, W4]), fdt),
        "W4Tg2": (_dup32(W4.T, 64), bdt),
        "W3Tg": (np.vstack([W3.T, W3.T]), bdt),
        "W2T": (W2.T, bdt),
        "Wl1bg": (_dup32(W1[:3, :], 128), bdt),
        "nones": (np.full((128, 1), -1.0), bdt),
        "b1e": ((b1 + float(t) * W1[3, :]).reshape(128, 1), np.float32),
        "b2": (b2.reshape(128, 1), np.float32),
        "b3g": (np.concatenate([b3, b3]).reshape(128, 1), np.float32),
        "b4": (b4.reshape(2, 1), np.float32),
    }
    return {("c_" + k): np.ascontiguousarray(v.astype(dt))
            for k, (v, dt) in c.items()}


def kernel(t, y, logp, v, W1, b1, W2, b2, W3, b3, W4, b4):
    global LAST_RESULTS
    del logp  # unused by the reference computation

    n = y.shape[0]
    assert n == N_FULL and n % N_CORES == 0
    ncc = n // N_CORES

    yT = np.ascontiguousarray(np.asarray(y, np.float32).T.astype(_np_dt(FWD_DT)))
    vT = np.ascontiguousarray(np.asarray(v, np.float32).T.astype(_np_dt(BWD_DT)))
    consts = _prep_consts(np.asarray(t).reshape(-1)[0], *[
        np.asarray(a, np.float32) for a in (W1, b1, W2, b2, W3, b3, W4, b4)])

    nc = _build(ncc, F, G)

    in_maps = []
    for c in range(N_CORES):
        sl = slice(c * ncc, (c + 1) * ncc)
        m = dict(consts)
        m["yT"] = np.ascontiguousarray(yT[:, sl])
        m["vT"] = np.ascontiguousarray(vT[:, sl])
        in_maps.append(m)

    trace = os.environ.get("CNF_TRACE", "0") == "1"
    res = run_bass_kernel_spmd(
        nc, in_maps, core_ids=list(range(N_CORES)), trace=trace)
    LAST_RESULTS = res

    dy = np.zeros((n, 3), dtype=np.float32)
    ndiv = np.empty((n, 1), dtype=np.float32)
    for c in range(N_CORES):
        sl = slice(c * ncc, (c + 1) * ncc)
        r = res.results[c]
        dy[sl, 0:2] = r["dyT"].T
        ndiv[sl, 0] = r["ndiv"][0]
    return dy, ndiv
